# revision 6
# baseline (speedup 1.0000x reference)
"""Trainium2 Bass kernel for the DeformableCurrents loss.

Energy e = e_ss - 2*e_st + e_tt where e_xy = sum_ij K(c_i, c_j) * <n_i, n_j>
with the Cauchy kernel K = 1/(1 + |ci - cj|^2).

Strategy (8-core SPMD, identical instruction stream per core, per-core data
staged by the host):
  - P-matmul (K=5 float32r):  P[j, i] = 1 + |y_j - x_i|^2 via augmented
    features, lhsT = feature block of 128 "j" points, rhs = feature chunk of
    512 "i" points -> PSUM [128, 512].
  - reciprocal: 3 of 4 units per group via DVE custom fast-reciprocal
    ([128,1536] in one op), 1 unit via ACT exp(-ln P). Output bf16.
  - S-matmul (K=128, M=3, bf16): S[d, i] += sum_j w*m[d,j] * Pinv[j,i],
    accumulated in PSUM over the 4 units of a pseudo-group. The symmetric
    doubling weight (and the -2 for e_st) is baked into the normals.
  - ACT copies S tiles out of PSUM; host computes sum_d,i n[d,i]*S[d,i].

Work decomposition: i-chunks of 512, j-blocks of 128. For the symmetric ss/tt
matrices only diagonal 512x512 super-blocks (weight 1) and strictly-upper
blocks (weight 2) are computed. Total units 2112 = 8 cores x 66 groups x 4.
"""

import numpy as np

V, N, M = 4096, 8192, 8192
CHUNK = 512
BLOCK = 128
NCORES = 8
PGS_PER_CORE = 66
UNITS_PER_PG = 4
_ACTIVE_PGS = None  # test hook: if set, only this many pgs are emitted
_REPEAT = 1         # test hook: emit the whole pg loop this many times
_LOOP_R = None      # test hook: wrap the body in a device-side For_i loop
_STAGE_MODE = "full"  # test hook: full | noegress | nomms | mmp

_CACHED_NC = None


# ---------------------------------------------------------------- planning
def _plan():
    """Global ordered list of 528 pseudo-groups (matrix, chunk, blocks[4], w[4])."""
    pgs = []
    for m in ("ss", "tt", "st"):
        for c in range(16):
            if m == "st":
                blocks = [(b, -2.0) for b in range(64)]
            else:
                blocks = [(b, 1.0) for b in range(4 * c, 4 * c + 4)]
                blocks += [(b, 2.0) for b in range(4 * c + 4, 64)]
            for k in range(0, len(blocks), 4):
                quad = blocks[k : k + 4]
                pgs.append((m, c, [b for b, _ in quad], [w for _, w in quad]))
    assert len(pgs) == NCORES * PGS_PER_CORE
    return pgs


# ---------------------------------------------------------------- bass build
def _build_nc():
    global _CACHED_NC
    if _CACHED_NC is not None:
        return _CACHED_NC

    from contextlib import ExitStack

    import concourse.bass as bass
    import concourse.tile as tile
    from concourse import bacc, mybir
    from concourse.dve_ops import RECIP_APPROX_FAST_CONSTS, RECIPROCAL_APPROX_FAST

    F32 = mybir.dt.float32
    F32R = mybir.dt.float32r
    F16 = mybir.dt.float16
    BF16 = mybir.dt.bfloat16
    AF = mybir.ActivationFunctionType

    nc = bacc.Bacc("TRN2", target_bir_lowering=False, debug=False,
                   num_devices=NCORES)

    # fp16 features: fp32r matmuls don't count as PE activity for the HAM
    # clock gate, so mixing them in pins the whole PE at 1.2 GHz. An all-
    # 16-bit matmul stream warms to 2.4 GHz after ~3.4us. fp16's 10-bit
    # mantissa keeps the d^2 gram expansion error ~3e-4 (vs 2e-2 budget).
    wfeat_d = nc.dram_tensor("wfeat", [5, PGS_PER_CORE, 512], BF16,
                             kind="ExternalInput").ap()
    rhsf_d = nc.dram_tensor("rhsf", [5, PGS_PER_CORE, 512], BF16,
                            kind="ExternalInput").ap()
    wnrm_d = nc.dram_tensor("wnrm", [128, PGS_PER_CORE, 12], BF16,
                            kind="ExternalInput").ap()
    # S results packed at 32-aligned partition bases {0,32,64,96} x 17
    # column blocks so the final DMA is wide
    sout_d = nc.dram_tensor("sout", [99, 17 * 512], F32,
                            kind="ExternalOutput").ap()

    rc = RECIP_APPROX_FAST_CONSTS

    with tile.TileContext(nc) as tc, ExitStack() as ctx:
        stage = ctx.enter_context(tc.tile_pool(name="stage", bufs=3))
        piv = ctx.enter_context(tc.tile_pool(name="piv", bufs=2))
        outp = ctx.enter_context(tc.tile_pool(name="outp", bufs=1))
        dvePA = ctx.enter_context(
            tc.tile_pool(name="dvePA", bufs=2, space=bass.MemorySpace.PSUM))
        dvePB = ctx.enter_context(
            tc.tile_pool(name="dvePB", bufs=1, space=bass.MemorySpace.PSUM))
        sP = ctx.enter_context(
            tc.tile_pool(name="sP", bufs=2, space=bass.MemorySpace.PSUM))

        mode = _STAGE_MODE
        sink = outp.tile([1, 64], F32, tag="sink")
        if mode == "full":
            sout = outp.tile([99, 17 * 512], F32, tag="sout")
        else:
            sout = None

        prev = None       # (pidB, pidA, wnrm_s, p) of previous pg
        pending = []      # [(s3_t, p)] egresses delayed by one more slot

        def emit_mms(prev):
            # S matmuls of the previous pg (PE stream, after this pg's MMPs)
            pidB, pidA, wnrm_s, p = prev
            s3_t = sP.tile([3, 512], F32, tag="s3")
            for k in range(2):
                nc.tensor.matmul(s3_t[:], wnrm_s[:, 3 * k : 3 * (k + 1)],
                                 pidB[:, 512 * k : 512 * (k + 1)],
                                 start=(k == 0), stop=False)
            for k in range(2):
                nc.tensor.matmul(s3_t[:], wnrm_s[:, 3 * (k + 2) : 3 * (k + 3)],
                                 pidA[:, 512 * k : 512 * (k + 1)],
                                 start=False, stop=(k == 1))
            return s3_t

        def emit_egress(s3_t, p):
            if _STAGE_MODE == "noegress":
                nc.vector.tensor_copy(sink[:, 32:36], s3_t[0:1, 0:4])
                return
            r, cblk = p % 4, p // 4
            nc.scalar.activation(
                sout[32 * r : 32 * r + 3, 512 * cblk : 512 * (cblk + 1)],
                s3_t[:], AF.Copy)

        SGB = 6  # pgs per staged DMA batch
        n_active = _ACTIVE_PGS if _ACTIVE_PGS is not None else PGS_PER_CORE

        from contextlib import nullcontext
        loop_cm = (tc.For_i(0, _LOOP_R, 1) if _LOOP_R else nullcontext())
        with loop_cm:
          for p0 in range(n_active * _REPEAT):
            p = p0 % n_active
            if p % SGB == 0:
                wfeat_t = stage.tile([5, SGB, 512], BF16, tag="wfeat")
                nc.sync.dma_start(wfeat_t[:], wfeat_d[:, p : p + SGB, :])
                rhsf_t = stage.tile([5, SGB, 512], BF16, tag="rhsf")
                nc.gpsimd.dma_start(rhsf_t[:], rhsf_d[:, p : p + SGB, :])
                wnrm_t = stage.tile([128, SGB, 12], BF16, tag="wnrm")
                nc.gpsimd.dma_start(wnrm_t[:], wnrm_d[:, p : p + SGB, :])
            s = p % SGB
            wfeat_s = wfeat_t[:, s, :]
            rhsf_s = rhsf_t[:, s, :]
            wnrm_s = wnrm_t[:, s, :]

            # ---- P matmuls: u0/u1 -> dvePB halves, u2/u3 -> dvePA halves
            dve_psB = dvePB.tile([128, 1024], F32, tag="dvepsB")
            for k in range(2):
                nc.tensor.matmul(dve_psB[:, 512 * k : 512 * (k + 1)],
                                 wfeat_s[:, 128 * k : 128 * (k + 1)],
                                 rhsf_s[:], start=True, stop=True)
            dve_psA = dvePA.tile([128, 1024], F32, tag="dvepsA")
            for k in range(2):
                nc.tensor.matmul(dve_psA[:, 512 * k : 512 * (k + 1)],
                                 wfeat_s[:, 128 * (k + 2) : 128 * (k + 3)],
                                 rhsf_s[:], start=True, stop=True)

            if mode == "mmp":
                nc.vector.tensor_copy(sink[:, 4:8], dve_psB[0:1, 0:4])
                nc.vector.tensor_copy(sink[:, 8:12], dve_psA[0:1, 0:4])
                continue

            # ---- reciprocals (all on DVE; ACT does only egress copies)
            pidB = piv.tile([128, 1024], BF16, tag="pidB")
            nc.vector._custom_dve(RECIPROCAL_APPROX_FAST, out=pidB[:],
                                  in0=dve_psB[:], s0=rc["s0"], s1=rc["s1"],
                                  imm2=rc["imm2"])
            pidA = piv.tile([128, 1024], BF16, tag="pidA")
            nc.vector._custom_dve(RECIPROCAL_APPROX_FAST, out=pidA[:],
                                  in0=dve_psA[:], s0=rc["s0"], s1=rc["s1"],
                                  imm2=rc["imm2"])

            if mode == "nomms":
                nc.vector.tensor_copy(sink[:, 20:24], pidB[0:1, 0:4])
                nc.vector.tensor_copy(sink[:, 24:28], pidA[0:1, 0:4])
                continue

            # ---- previous pg's S matmuls follow this pg's P matmuls in the
            # PE stream (PE never waits on this pg's reciprocals); egresses
            # are delayed one further slot so ACT never waits on MMS
            if prev is not None:
                pending.append((emit_mms(prev), prev[3]))
            if len(pending) > 1:
                emit_egress(*pending.pop(0))

            prev = (pidB, pidA, wnrm_s, p)

          # pipeline flush (inside the optional timing loop: body self-contained)
          if prev is not None:
              pending.append((emit_mms(prev), prev[3]))
              for item in pending:
                  emit_egress(*item)
          prev = None
          pending = []

        if mode == "full":
            nc.sync.dma_start(sout_d[:], sout[:])
        else:
            nc.sync.dma_start(sout_d[0:1, 0:64], sink[:])

    nc.compile()
    _CACHED_NC = nc
    return nc


# ---------------------------------------------------------------- host side
def _feats(pts):
    """pts [n,3] f32 -> featL [5,n] (lhsT side), featR [5,n] (rhs side)."""
    x, y, z = pts[:, 0], pts[:, 1], pts[:, 2]
    n2 = x * x + y * y + z * z
    one = np.ones_like(n2)
    featL = np.stack([x, y, z, n2, one]).astype(np.float32)
    featR = np.stack([-2 * x, -2 * y, -2 * z, one, n2 + 1.0]).astype(np.float32)
    return featL, featR


def kernel(src_vertices, tar_normals, tar_centers, src_indices):
    import ml_dtypes
    from concourse.bass_utils import run_bass_kernel_spmd

    src_vertices = np.asarray(src_vertices, dtype=np.float32)
    tar_normals = np.asarray(tar_normals, dtype=np.float32)
    tar_centers = np.asarray(tar_centers, dtype=np.float32)
    idx = np.asarray(src_indices).astype(np.int64)

    # triangle gather: normals and centers of source triangles
    tris = src_vertices[idx]                      # [N, 3, 3]
    a, b, c = tris[:, 0, :], tris[:, 1, :], tris[:, 2, :]
    normals = 0.5 * np.cross(a - b, c - b).astype(np.float32)   # [N,3]
    centers = (tris.sum(axis=1) / 3.0).astype(np.float32)       # [N,3]

    sfL, sfR = _feats(centers)
    tfL, tfR = _feats(tar_centers)
    snT = normals.T.astype(np.float64)        # [3, N] finalize side
    tnT = tar_normals.T.astype(np.float64)

    featL = {"ss": sfL, "tt": tfL, "st": tfL}   # partition (j) side
    featR = {"ss": sfR, "tt": tfR, "st": sfR}   # free (i) side
    nrmP = {"ss": normals, "tt": tar_normals, "st": tar_normals}  # [n,3] j side
    fnT = {"ss": snT, "tt": tnT, "st": snT}     # [3,n] i side (host)

    pgs = _plan()
    in_maps = []
    fn_slices = []  # per core, per pg: [3,512] f64 host-side finalize normals
    for core in range(NCORES):
        my = pgs[core * PGS_PER_CORE : (core + 1) * PGS_PER_CORE]
        wfeat = np.empty((PGS_PER_CORE, 5, 512), np.float32)
        rhsf = np.empty((PGS_PER_CORE, 5, 512), np.float32)
        wnrm = np.empty((PGS_PER_CORE, 128, 12), np.float32)
        fns = []
        for p, (m, cch, blocks, ws) in enumerate(my):
            rhsf[p] = featR[m][:, CHUNK * cch : CHUNK * (cch + 1)]
            for q, (blk, w) in enumerate(zip(blocks, ws)):
                wfeat[p, :, 128 * q : 128 * (q + 1)] = (
                    featL[m][:, BLOCK * blk : BLOCK * (blk + 1)])
                wnrm[p, :, 3 * q : 3 * (q + 1)] = (
                    w * nrmP[m][BLOCK * blk : BLOCK * (blk + 1), :])
            fns.append(fnT[m][:, CHUNK * cch : CHUNK * (cch + 1)])
        in_maps.append({
            "wfeat": np.ascontiguousarray(
                wfeat.transpose(1, 0, 2)).astype(ml_dtypes.bfloat16),
            "rhsf": np.ascontiguousarray(
                rhsf.transpose(1, 0, 2)).astype(ml_dtypes.bfloat16),
            "wnrm": np.ascontiguousarray(
                wnrm.transpose(1, 0, 2)).astype(ml_dtypes.bfloat16),
        })
        fn_slices.append(fns)

    nc = _build_nc()
    results = run_bass_kernel_spmd(nc, in_maps, list(range(NCORES))).results

    e = 0.0
    for core in range(NCORES):
        sout = np.asarray(results[core]["sout"], dtype=np.float64)  # [99, 17*512]
        for p in range(PGS_PER_CORE):
            r, cblk = p % 4, p // 4
            S = sout[32 * r : 32 * r + 3, 512 * cblk : 512 * (cblk + 1)]
            e += float((S * fn_slices[core][p]).sum())
    return np.float32(e)



# revision 11
# speedup vs baseline: 1.6541x; 1.6541x over previous
"""Trainium2 Bass kernel for the DeformableCurrents loss (super-tile version).

Energy e = e_ss - 2*e_st + e_tt where e_xy = sum_ij K(c_i, c_j) * <n_i, n_j>
with the Cauchy kernel K = 1/(1 + |ci - cj|^2).

Work decomposition: "visits" of 4 consecutive j-blocks (128 rows each) x up
to 4 i-chunks (512 cols each). Per visit, each block-step shares one P
stationary (features of the block) across its chunks and one S stationary
(normals of the block); S results accumulate in PSUM across the visit's 4
blocks, then two stacked [3,512]-pair accumulator banks are DMA'd straight
to DRAM. Host does the tiny weighted finalize dot.

Key HW facts this shape exploits (measured on these cores):
  - Matmuls with K<=64, or in fp32r/fp16, don't register as PE activity for
    the HAM clock gate -> whole kernel runs at 1.2 GHz. All-bf16 K=128
    streams warm to 2.4 GHz. Features are therefore zero-padded to K=128.
  - Split-bf16 (hi+lo) feature expansion gives ~fp32-grade d^2 at no PE
    cost (13 of the 128 stationary rows are real, rest zeros).
  - The DVE reciprocal is the co-bottleneck; 640 of each step's 2048
    columns go to ACT as exp(-ln P) instead (one act table holds Ln/Exp).
"""

import numpy as np

V, N, M = 4096, 8192, 8192
CHUNK = 512
BLOCK = 128
NCORES = 8
VISITS_PER_CORE = 18
NFEAT = 13
_CACHED_NC = None

# ACT takes this many of the last A-tile columns per full block-step
ACT_COLS = 704


# ---------------------------------------------------------------- planning
def _plan2():
    """Global visit list. Visit = (matrix, blocks[4], chunks[k], weights[k]).

    ss/tt use the upper triangle: block-group g (blocks 16g..16g+15) sees
    full 4x4-chunk supers against every quad q<g, plus a diagonal staircase
    of 4x(k+1) tiles (k=0..3) where the last chunk is the diagonal (w=1)
    and earlier quad chunks are strictly-upper (w=2). st is dense, w=-2.
    """
    fours, threes, twos, ones = [], [], [], []
    by_len = {4: fours, 3: threes, 2: twos, 1: ones}
    for m in ("ss", "tt"):
        for g in range(4):
            for q in range(g):
                for k in range(4):
                    blocks = [16 * g + 4 * k + j for j in range(4)]
                    chunks = list(range(4 * q, 4 * q + 4))
                    by_len[4].append((m, blocks, chunks, [2.0] * 4))
            for k in range(4):
                blocks = [16 * g + 4 * k + j for j in range(4)]
                chunks = list(range(4 * g, 4 * g + k + 1))
                w = [2.0] * k + [1.0]
                by_len[k + 1].append((m, blocks, chunks, w))
    for bg in range(16):
        for q in range(4):
            blocks = [4 * bg + j for j in range(4)]
            chunks = list(range(4 * q, 4 * q + 4))
            fours.append(("st", blocks, chunks, [-2.0] * 4))
    assert len(fours) == 120 and len(threes) == 8 and len(twos) == 8 \
        and len(ones) == 8
    cores = []
    for c in range(NCORES):
        f = fours[15 * c : 15 * c + 15]
        # ragged visits interleaved mid-stream (not as a pipeline-draining
        # tail); positions must match _VISIT_K on every core
        cores.append(f[0:5] + [threes[c]] + f[5:10] + [twos[c]]
                     + f[10:15] + [ones[c]])
    return cores


# visit k-shape sequence, identical for every core
_VISIT_K = [4] * 5 + [3] + [4] * 5 + [2] + [4] * 5 + [1]


# ---------------------------------------------------------------- bass build
def _build_nc():
    global _CACHED_NC
    if _CACHED_NC is not None:
        return _CACHED_NC

    from contextlib import ExitStack

    import concourse.bass as bass
    import concourse.tile as tile
    from concourse import bacc, mybir
    from concourse.dve_ops import RECIP_APPROX_FAST_CONSTS, RECIPROCAL_APPROX_FAST

    F32 = mybir.dt.float32
    F16 = mybir.dt.float16
    BF16 = mybir.dt.bfloat16
    AF = mybir.ActivationFunctionType

    nc = bacc.Bacc("TRN2", target_bir_lowering=False, debug=False,
                   num_devices=NCORES)

    # Pin Ln/Exp/Copy to the one table set containing all three so the
    # table-load fixpoint emits a single LoadActFuncSet instead of swapping
    # sets between the exp(-ln) reciprocals and the egress copies (~1.3us
    # per swap, and the resulting PE starvation re-chills the HAM gate).
    from concourse.hw_specs import get_activation_tables
    _tabs = get_activation_tables(nc.m.arch)
    _pinned = {AF.Ln, AF.Exp, AF.Copy}
    if "natural_log_exp_and_others" in _tabs:
        for _name, _fns in _tabs.items():
            if _name != "natural_log_exp_and_others":
                _fns -= _pinned

    wfeat_d = nc.dram_tensor("wfeat", [NFEAT, VISITS_PER_CORE, 4, 128], BF16,
                             kind="ExternalInput").ap()
    rhsf_d = nc.dram_tensor("rhsf", [NFEAT, VISITS_PER_CORE, 4, 512], BF16,
                            kind="ExternalInput").ap()
    wnrm_d = nc.dram_tensor("wnrm", [128, VISITS_PER_CORE, 12], BF16,
                            kind="ExternalInput").ap()
    sout_d = nc.dram_tensor("sout", [VISITS_PER_CORE, 4, 3, 512], F32,
                            kind="ExternalOutput").ap()

    rc = RECIP_APPROX_FAST_CONSTS
    SGB = 2  # visits per staged DMA batch

    with tile.TileContext(nc) as tc, ExitStack() as ctx:
        stage = ctx.enter_context(tc.tile_pool(name="stage", bufs=3))
        piv = ctx.enter_context(tc.tile_pool(name="piv", bufs=3))
        outp = ctx.enter_context(tc.tile_pool(name="outp", bufs=1))
        # PSUM: pB 2 banks x1 (DVE reads it early each step), pA 2 banks x2
        # (double-buffered because its ACT Ln reader runs a step late), acc
        # (four [3,512] slots stacked at partition offsets 0/32/64/96) x2 = 8
        pBp = ctx.enter_context(
            tc.tile_pool(name="pBp", bufs=1, space=bass.MemorySpace.PSUM))
        pAp = ctx.enter_context(
            tc.tile_pool(name="pAp", bufs=2, space=bass.MemorySpace.PSUM))
        accp = ctx.enter_context(
            tc.tile_pool(name="accp", bufs=2, space=bass.MemorySpace.PSUM))

        # K=128 zero-padded feature staging (rows NFEAT..127 stay zero):
        # K<=64 matmuls don't count as PE activity for the HAM clock gate.
        wfp = [outp.tile([128, SGB, 4, 128], BF16, tag=f"wfp{i}",
                         name=f"wfp{i}") for i in range(2)]
        rfp = [outp.tile([128, SGB, 4, 512], BF16, tag=f"rfp{i}",
                         name=f"rfp{i}") for i in range(2)]
        # Warm the HAM clock gate during the staging prologue: throwaway
        # K=128 matmuls on a tiny dedicated zero tile keep the PE busy past
        # the ~3.4us activity window, so the real stream starts at 2.4 GHz
        # instead of paying ~15us of half-rate matmuls. The dedicated tile
        # (memset first, on the fast DVE) keeps the warm-up off the staging
        # buffers' dependency chains.
        wz = outp.tile([128, 512], BF16, tag="wz", name="wz")
        nc.vector.memset(wz[:], 0.0)
        warm = pBp.tile([128, 1024], F32, tag="pB", name="pBwarm")
        for _ in range(10):
            nc.tensor.matmul(warm[:, 0:512], wz[:, 0:128], wz[:],
                             start=True, stop=True)

        nc.vector.memset(wfp[0][:], 0.0)
        nc.gpsimd.memset(wfp[1][:], 0.0)
        nc.vector.memset(rfp[0][:], 0.0)
        nc.gpsimd.memset(rfp[1][:], 0.0)

        NSTEP = 4 * VISITS_PER_CORE  # 4 block-steps per visit
        prevq = []      # step contexts: Exp at age 1, S matmuls at age 2
        pending = []    # completed visits awaiting egress DMA
        acc = {}        # live accumulators of the current visit

        def emit_exp(pv):
            if pv[6] is not None:
                nc.scalar.activation(pv[1][:, 1024 - ACT_COLS : 1024],
                                     pv[6][:], AF.Exp, scale=-1.0)

        def emit_s(pv):
            (pidB, pidA, wn, k, v, t, _lnt) = pv
            if t == 0:
                acc["A"] = accp.tile([99, 512], F32, tag="acc", name="acc")
            start, stop = (t == 0), (t == 3)
            at = acc["A"]
            for c in range(k):
                off = 32 * c
                mv = pidB[:, 512 * c : 512 * (c + 1)] if c < 2 else \
                    pidA[:, 512 * (c - 2) : 512 * (c - 1)]
                nc.tensor.matmul(at[off : off + 3], wn, mv,
                                 start=start, stop=stop,
                                 tile_position=(0, off))
            if stop:
                pending.append((at, v, k))

        def emit_egress():
            at, v, k = pending.pop(0)
            # DMA (and gpsimd) cannot read PSUM; bounce through SBUF via one
            # wide ACT copy (partition-parallel: [99,512] costs same as
            # [3,512]).
            eg = piv.tile([99, 512], F32, tag="eg")
            nc.scalar.activation(eg[:], at[:], AF.Copy)
            for c in range(k):
                nc.sync.dma_start(sout_d[v, c], eg[32 * c : 32 * c + 3])

        for u in range(NSTEP):
            v, t = u // 4, u % 4
            k = _VISIT_K[v]
            bi = (v // SGB) % 2
            s = v % SGB
            if t == 0 and s == 0:
                nc.sync.dma_start(wfp[bi][0:NFEAT],
                                  wfeat_d[:, v : v + SGB])
                nc.gpsimd.dma_start(rfp[bi][0:NFEAT],
                                    rhsf_d[:, v : v + SGB])
                wnp_t = stage.tile([128, SGB, 12], BF16, tag="wnp")
                nc.gpsimd.dma_start(wnp_t[:], wnrm_d[:, v : v + SGB])

            # ---- P matmuls for this block-step (chunks 0,1 -> pB; 2,3 -> pA)
            lhs = wfp[bi][:, s, t, :]
            pB = pBp.tile([128, 1024], F32, tag="pB", name="pB")
            pA = pAp.tile([128, 1024], F32, tag="pA", name="pA") if k > 2 \
                else None
            for c in range(k):
                dst = pB[:, 512 * c : 512 * (c + 1)] if c < 2 else \
                    pA[:, 512 * (c - 2) : 512 * (c - 1)]
                nc.tensor.matmul(dst, lhs, rfp[bi][:, s, c, :],
                                 start=True, stop=True)

            # ---- reciprocals: DVE takes pB + the head of pA, ACT the tail
            pidB = piv.tile([128, 1024], BF16, tag="pidB")
            nc.vector._custom_dve(RECIPROCAL_APPROX_FAST,
                                  out=pidB[:, : 512 * min(k, 2)],
                                  in0=pB[:, : 512 * min(k, 2)],
                                  s0=rc["s0"], s1=rc["s1"], imm2=rc["imm2"])
            pidA = None
            lnt = None
            if k > 2:
                pidA = piv.tile([128, 1024], BF16, tag="pidA")
                na = 512 * (k - 2)
                dv = na - ACT_COLS if k == 4 else na
                nc.vector._custom_dve(RECIPROCAL_APPROX_FAST,
                                      out=pidA[:, :dv], in0=pA[:, :dv],
                                      s0=rc["s0"], s1=rc["s1"],
                                      imm2=rc["imm2"])

            # ACT queue: Exp(u-1) first (its S consumer is imminent),
            # Ln(u) after (pA double-buffered -> a full step of slack).
            if prevq:
                emit_exp(prevq[0])
            if k == 4:
                lnt = piv.tile([128, ACT_COLS], F16, tag="lnt")
                nc.scalar.activation(lnt[:], pA[:, dv:na], AF.Ln)

            # ---- previous step's Exp + S matmuls follow this step's work.
            # ACT queue order matters: Exp(u-1) goes FIRST (its S consumer
            # is imminent), Ln(u) after (pA is double-buffered, so its
            # reader has a full step of slack).
            if prevq:
                emit_s(prevq.pop(0))
            if len(pending) > 1:
                emit_egress()
            prevq.append((pidB, pidA, wnp_t[:, s, 3 * t : 3 * t + 3],
                          k, v, t, lnt))

        while prevq:
            emit_exp(prevq[0])
            emit_s(prevq.pop(0))
        while pending:
            emit_egress()

    nc.compile()
    _CACHED_NC = nc
    return nc


# ---------------------------------------------------------------- host side
def _split(v):
    """f32 array -> (hi, lo) bf16-representable f32 parts."""
    import ml_dtypes
    hi = v.astype(ml_dtypes.bfloat16).astype(np.float32)
    lo = (v - hi).astype(ml_dtypes.bfloat16).astype(np.float32)
    return hi, lo


def _feats13(pts):
    """pts [n,3] f32 -> featL [13,n], featR [13,n] f32 (bf16-exact values).

    Split-bf16 gram features: d^2+1 = sum_k L_k R_k with each coordinate
    cross term expanded as xh*Xh + xh*Xl + xl*Xh (X = -2x')."""
    x, y, z = pts[:, 0], pts[:, 1], pts[:, 2]
    n2 = x * x + y * y + z * z
    one = np.ones_like(n2)
    L, R = [], []
    for c in (x, y, z):
        ch, cl = _split(c)
        Xh, Xl = _split(-2.0 * c)
        L += [ch, ch, cl]
        R += [Xh, Xl, Xh]
    n2h, n2l = _split(n2)
    m2h, m2l = _split(n2 + 1.0)
    L += [n2h, n2l, one, one]
    R += [one, one, m2h, m2l]
    return (np.stack(L).astype(np.float32), np.stack(R).astype(np.float32))


def kernel(src_vertices, tar_normals, tar_centers, src_indices):
    import ml_dtypes
    from concourse.bass_utils import run_bass_kernel_spmd

    src_vertices = np.asarray(src_vertices, dtype=np.float32)
    tar_normals = np.asarray(tar_normals, dtype=np.float32)
    tar_centers = np.asarray(tar_centers, dtype=np.float32)
    idx = np.asarray(src_indices).astype(np.int64)

    tris = src_vertices[idx]                      # [N, 3, 3]
    a, b, c = tris[:, 0, :], tris[:, 1, :], tris[:, 2, :]
    normals = 0.5 * np.cross(a - b, c - b).astype(np.float32)   # [N,3]
    centers = (tris.sum(axis=1) / 3.0).astype(np.float32)       # [N,3]

    sfL, sfR = _feats13(centers)
    tfL, tfR = _feats13(tar_centers)
    snT = normals.T.astype(np.float64)
    tnT = tar_normals.T.astype(np.float64)

    featL = {"ss": sfL, "tt": tfL, "st": tfL}   # stationary (j) side
    featR = {"ss": sfR, "tt": tfR, "st": sfR}   # moving (i) side
    nrmP = {"ss": normals, "tt": tar_normals, "st": tar_normals}
    fnT = {"ss": snT, "tt": tnT, "st": snT}     # finalize (i) side

    plans = _plan2()
    in_maps = []
    fin = []   # per core: list of (visit, slot, w, fn [3,512] f64)
    for core in range(NCORES):
        visits = plans[core]
        wfeat = np.zeros((NFEAT, VISITS_PER_CORE, 4, 128), np.float32)
        rhsf = np.zeros((NFEAT, VISITS_PER_CORE, 4, 512), np.float32)
        wnrm = np.zeros((128, VISITS_PER_CORE, 12), np.float32)
        fv = []
        for vi, (m, blocks, chunks, ws) in enumerate(visits):
            for tj, blk in enumerate(blocks):
                wfeat[:, vi, tj, :] = featL[m][:, BLOCK * blk : BLOCK * (blk + 1)]
                wnrm[:, vi, 3 * tj : 3 * tj + 3] = (
                    nrmP[m][BLOCK * blk : BLOCK * (blk + 1), :])
            for cj, (ch, w) in enumerate(zip(chunks, ws)):
                rhsf[:, vi, cj, :] = featR[m][:, CHUNK * ch : CHUNK * (ch + 1)]
                fv.append((vi, cj, w,
                           fnT[m][:, CHUNK * ch : CHUNK * (ch + 1)]))
        in_maps.append({
            "wfeat": wfeat.astype(ml_dtypes.bfloat16),
            "rhsf": rhsf.astype(ml_dtypes.bfloat16),
            "wnrm": wnrm.astype(ml_dtypes.bfloat16),
        })
        fin.append(fv)

    nc = _build_nc()
    results = run_bass_kernel_spmd(nc, in_maps, list(range(NCORES))).results

    e = 0.0
    for core in range(NCORES):
        sout = np.asarray(results[core]["sout"], dtype=np.float64)
        for vi, cj, w, fn in fin[core]:
            e += w * float((sout[vi, cj] * fn).sum())
    return np.float32(e)


# revision 12
# speedup vs baseline: 1.6655x; 1.0069x over previous
"""Trainium2 Bass kernel for the DeformableCurrents loss (super-tile version).

Energy e = e_ss - 2*e_st + e_tt where e_xy = sum_ij K(c_i, c_j) * <n_i, n_j>
with the Cauchy kernel K = 1/(1 + |ci - cj|^2).

Work decomposition: "visits" of 4 consecutive j-blocks (128 rows each) x up
to 4 i-chunks (512 cols each). Per visit, each block-step shares one P
stationary (features of the block) across its chunks and one S stationary
(normals of the block); S results accumulate in PSUM across the visit's 4
blocks, then two stacked [3,512]-pair accumulator banks are DMA'd straight
to DRAM. Host does the tiny weighted finalize dot.

Key HW facts this shape exploits (measured on these cores):
  - Matmuls with K<=64, or in fp32r/fp16, don't register as PE activity for
    the HAM clock gate -> whole kernel runs at 1.2 GHz. All-bf16 K=128
    streams warm to 2.4 GHz. Features are therefore zero-padded to K=128.
  - Split-bf16 (hi+lo) feature expansion gives ~fp32-grade d^2 at no PE
    cost (13 of the 128 stationary rows are real, rest zeros).
  - The DVE reciprocal is the co-bottleneck; 640 of each step's 2048
    columns go to ACT as exp(-ln P) instead (one act table holds Ln/Exp).
"""

import numpy as np

V, N, M = 4096, 8192, 8192
CHUNK = 512
BLOCK = 128
NCORES = 8
VISITS_PER_CORE = 18
NFEAT = 13
_CACHED_NC = None

# ACT takes this many of the last A-tile columns per full block-step
ACT_COLS = 704


# ---------------------------------------------------------------- planning
def _plan2():
    """Global visit list. Visit = (matrix, blocks[4], chunks[k], weights[k]).

    ss/tt use the upper triangle: block-group g (blocks 16g..16g+15) sees
    full 4x4-chunk supers against every quad q<g, plus a diagonal staircase
    of 4x(k+1) tiles (k=0..3) where the last chunk is the diagonal (w=1)
    and earlier quad chunks are strictly-upper (w=2). st is dense, w=-2.
    """
    fours, threes, twos, ones = [], [], [], []
    by_len = {4: fours, 3: threes, 2: twos, 1: ones}
    for m in ("ss", "tt"):
        for g in range(4):
            for q in range(g):
                for k in range(4):
                    blocks = [16 * g + 4 * k + j for j in range(4)]
                    chunks = list(range(4 * q, 4 * q + 4))
                    by_len[4].append((m, blocks, chunks, [2.0] * 4))
            for k in range(4):
                blocks = [16 * g + 4 * k + j for j in range(4)]
                chunks = list(range(4 * g, 4 * g + k + 1))
                w = [2.0] * k + [1.0]
                by_len[k + 1].append((m, blocks, chunks, w))
    for bg in range(16):
        for q in range(4):
            blocks = [4 * bg + j for j in range(4)]
            chunks = list(range(4 * q, 4 * q + 4))
            fours.append(("st", blocks, chunks, [-2.0] * 4))
    assert len(fours) == 120 and len(threes) == 8 and len(twos) == 8 \
        and len(ones) == 8
    cores = []
    for c in range(NCORES):
        f = fours[15 * c : 15 * c + 15]
        # ragged visits interleaved mid-stream (not as a pipeline-draining
        # tail); positions must match _VISIT_K on every core
        cores.append(f[0:5] + [threes[c]] + f[5:10] + [twos[c]]
                     + f[10:15] + [ones[c]])
    return cores


# visit k-shape sequence, identical for every core
_VISIT_K = [4] * 5 + [3] + [4] * 5 + [2] + [4] * 5 + [1]


# ---------------------------------------------------------------- bass build
def _build_nc():
    global _CACHED_NC
    if _CACHED_NC is not None:
        return _CACHED_NC

    from contextlib import ExitStack

    import concourse.bass as bass
    import concourse.tile as tile
    from concourse import bacc, mybir
    from concourse.dve_ops import RECIP_APPROX_FAST_CONSTS, RECIPROCAL_APPROX_FAST

    F32 = mybir.dt.float32
    F16 = mybir.dt.float16
    BF16 = mybir.dt.bfloat16
    AF = mybir.ActivationFunctionType

    nc = bacc.Bacc("TRN2", target_bir_lowering=False, debug=False,
                   num_devices=NCORES)

    # Pin Ln/Exp/Copy to the one table set containing all three so the
    # table-load fixpoint emits a single LoadActFuncSet instead of swapping
    # sets between the exp(-ln) reciprocals and the egress copies (~1.3us
    # per swap, and the resulting PE starvation re-chills the HAM gate).
    from concourse.hw_specs import get_activation_tables
    _tabs = get_activation_tables(nc.m.arch)
    _pinned = {AF.Ln, AF.Exp, AF.Copy}
    if "natural_log_exp_and_others" in _tabs:
        for _name, _fns in _tabs.items():
            if _name != "natural_log_exp_and_others":
                _fns -= _pinned

    wfeat_d = nc.dram_tensor("wfeat", [NFEAT, VISITS_PER_CORE, 4, 128], BF16,
                             kind="ExternalInput").ap()
    rhsf_d = nc.dram_tensor("rhsf", [NFEAT, VISITS_PER_CORE, 4, 512], BF16,
                            kind="ExternalInput").ap()
    wnrm_d = nc.dram_tensor("wnrm", [128, VISITS_PER_CORE, 12], BF16,
                            kind="ExternalInput").ap()
    # [99,512] per visit: rows 32c..32c+2 hold slot c; one wide DMA per
    # visit beats four narrow partition-sliced ones (trigger-bound tail)
    sout_d = nc.dram_tensor("sout", [VISITS_PER_CORE, 99, 512], F32,
                            kind="ExternalOutput").ap()

    rc = RECIP_APPROX_FAST_CONSTS
    SGB = 2  # visits per staged DMA batch

    with tile.TileContext(nc) as tc, ExitStack() as ctx:
        stage = ctx.enter_context(tc.tile_pool(name="stage", bufs=3))
        piv = ctx.enter_context(tc.tile_pool(name="piv", bufs=3))
        outp = ctx.enter_context(tc.tile_pool(name="outp", bufs=1))
        # PSUM: pB 2 banks x1 (DVE reads it early each step), pA 2 banks x2
        # (double-buffered because its ACT Ln reader runs a step late), acc
        # (four [3,512] slots stacked at partition offsets 0/32/64/96) x2 = 8
        pBp = ctx.enter_context(
            tc.tile_pool(name="pBp", bufs=1, space=bass.MemorySpace.PSUM))
        pAp = ctx.enter_context(
            tc.tile_pool(name="pAp", bufs=2, space=bass.MemorySpace.PSUM))
        accp = ctx.enter_context(
            tc.tile_pool(name="accp", bufs=2, space=bass.MemorySpace.PSUM))

        # K=128 zero-padded feature staging (rows NFEAT..127 stay zero):
        # K<=64 matmuls don't count as PE activity for the HAM clock gate.
        wfp = [outp.tile([128, SGB, 4, 128], BF16, tag=f"wfp{i}",
                         name=f"wfp{i}") for i in range(2)]
        rfp = [outp.tile([128, SGB, 4, 512], BF16, tag=f"rfp{i}",
                         name=f"rfp{i}") for i in range(2)]
        # Warm the HAM clock gate during the staging prologue: throwaway
        # K=128 matmuls on a tiny dedicated zero tile keep the PE busy past
        # the ~3.4us activity window, so the real stream starts at 2.4 GHz
        # instead of paying ~15us of half-rate matmuls. The dedicated tile
        # (memset first, on the fast DVE) keeps the warm-up off the staging
        # buffers' dependency chains.
        wz = outp.tile([128, 512], BF16, tag="wz", name="wz")
        nc.vector.memset(wz[:], 0.0)
        warm = pBp.tile([128, 1024], F32, tag="pB", name="pBwarm")
        for _ in range(25):
            nc.tensor.matmul(warm[:, 0:512], wz[:, 0:128], wz[:],
                             start=True, stop=True)

        nc.vector.memset(wfp[0][:], 0.0)
        nc.gpsimd.memset(wfp[1][:], 0.0)
        nc.vector.memset(rfp[0][:], 0.0)
        nc.gpsimd.memset(rfp[1][:], 0.0)

        NSTEP = 4 * VISITS_PER_CORE  # 4 block-steps per visit
        prevq = []      # step contexts: Exp at age 1, S matmuls at age 2
        pending = []    # completed visits awaiting egress DMA
        acc = {}        # live accumulators of the current visit

        def emit_exp(pv):
            if pv[6] is not None:
                nc.scalar.activation(pv[1][:, 1024 - ACT_COLS : 1024],
                                     pv[6][:], AF.Exp, scale=-1.0)

        def emit_s(pv):
            (pidB, pidA, wn, k, v, t, _lnt) = pv
            if t == 0:
                acc["A"] = accp.tile([99, 512], F32, tag="acc", name="acc")
            start, stop = (t == 0), (t == 3)
            at = acc["A"]
            for c in range(k):
                off = 32 * c
                mv = pidB[:, 512 * c : 512 * (c + 1)] if c < 2 else \
                    pidA[:, 512 * (c - 2) : 512 * (c - 1)]
                nc.tensor.matmul(at[off : off + 3], wn, mv,
                                 start=start, stop=stop,
                                 tile_position=(0, off))
            if stop:
                pending.append((at, v, k))

        def emit_egress():
            at, v, k = pending.pop(0)
            # DMA (and gpsimd) cannot read PSUM; bounce through SBUF via one
            # wide ACT copy (partition-parallel: [99,512] costs same as
            # [3,512]).
            eg = piv.tile([99, 512], F32, tag="eg")
            nc.scalar.activation(eg[:], at[:], AF.Copy)
            nc.sync.dma_start(sout_d[v], eg[:])

        for u in range(NSTEP):
            v, t = u // 4, u % 4
            k = _VISIT_K[v]
            bi = (v // SGB) % 2
            s = v % SGB
            if t == 0 and s == 0:
                nc.sync.dma_start(wfp[bi][0:NFEAT],
                                  wfeat_d[:, v : v + SGB])
                nc.gpsimd.dma_start(rfp[bi][0:NFEAT],
                                    rhsf_d[:, v : v + SGB])
                wnp_t = stage.tile([128, SGB, 12], BF16, tag="wnp")
                nc.gpsimd.dma_start(wnp_t[:], wnrm_d[:, v : v + SGB])

            # ---- P matmuls for this block-step (chunks 0,1 -> pB; 2,3 -> pA)
            lhs = wfp[bi][:, s, t, :]
            pB = pBp.tile([128, 1024], F32, tag="pB", name="pB")
            pA = pAp.tile([128, 1024], F32, tag="pA", name="pA") if k > 2 \
                else None
            for c in range(k):
                dst = pB[:, 512 * c : 512 * (c + 1)] if c < 2 else \
                    pA[:, 512 * (c - 2) : 512 * (c - 1)]
                nc.tensor.matmul(dst, lhs, rfp[bi][:, s, c, :],
                                 start=True, stop=True)

            # ---- reciprocals: DVE takes pB + the head of pA, ACT the tail
            pidB = piv.tile([128, 1024], BF16, tag="pidB")
            nc.vector._custom_dve(RECIPROCAL_APPROX_FAST,
                                  out=pidB[:, : 512 * min(k, 2)],
                                  in0=pB[:, : 512 * min(k, 2)],
                                  s0=rc["s0"], s1=rc["s1"], imm2=rc["imm2"])
            pidA = None
            lnt = None
            if k > 2:
                pidA = piv.tile([128, 1024], BF16, tag="pidA")
                na = 512 * (k - 2)
                dv = na - ACT_COLS if k == 4 else na
                nc.vector._custom_dve(RECIPROCAL_APPROX_FAST,
                                      out=pidA[:, :dv], in0=pA[:, :dv],
                                      s0=rc["s0"], s1=rc["s1"],
                                      imm2=rc["imm2"])

            # ACT queue: Exp(u-1) first (its S consumer is imminent),
            # Ln(u) after (pA double-buffered -> a full step of slack).
            if prevq:
                emit_exp(prevq[0])
            if k == 4:
                lnt = piv.tile([128, ACT_COLS], F16, tag="lnt")
                nc.scalar.activation(lnt[:], pA[:, dv:na], AF.Ln)

            # ---- previous step's Exp + S matmuls follow this step's work.
            # ACT queue order matters: Exp(u-1) goes FIRST (its S consumer
            # is imminent), Ln(u) after (pA is double-buffered, so its
            # reader has a full step of slack).
            if prevq:
                emit_s(prevq.pop(0))
            if len(pending) > 1:
                emit_egress()
            prevq.append((pidB, pidA, wnp_t[:, s, 3 * t : 3 * t + 3],
                          k, v, t, lnt))

        while prevq:
            emit_exp(prevq[0])
            emit_s(prevq.pop(0))
        while pending:
            emit_egress()

    nc.compile()
    _CACHED_NC = nc
    return nc


# ---------------------------------------------------------------- host side
def _split(v):
    """f32 array -> (hi, lo) bf16-representable f32 parts."""
    import ml_dtypes
    hi = v.astype(ml_dtypes.bfloat16).astype(np.float32)
    lo = (v - hi).astype(ml_dtypes.bfloat16).astype(np.float32)
    return hi, lo


def _feats13(pts):
    """pts [n,3] f32 -> featL [13,n], featR [13,n] f32 (bf16-exact values).

    Split-bf16 gram features: d^2+1 = sum_k L_k R_k with each coordinate
    cross term expanded as xh*Xh + xh*Xl + xl*Xh (X = -2x')."""
    x, y, z = pts[:, 0], pts[:, 1], pts[:, 2]
    n2 = x * x + y * y + z * z
    one = np.ones_like(n2)
    L, R = [], []
    for c in (x, y, z):
        ch, cl = _split(c)
        Xh, Xl = _split(-2.0 * c)
        L += [ch, ch, cl]
        R += [Xh, Xl, Xh]
    n2h, n2l = _split(n2)
    m2h, m2l = _split(n2 + 1.0)
    L += [n2h, n2l, one, one]
    R += [one, one, m2h, m2l]
    return (np.stack(L).astype(np.float32), np.stack(R).astype(np.float32))


def kernel(src_vertices, tar_normals, tar_centers, src_indices):
    import ml_dtypes
    from concourse.bass_utils import run_bass_kernel_spmd

    src_vertices = np.asarray(src_vertices, dtype=np.float32)
    tar_normals = np.asarray(tar_normals, dtype=np.float32)
    tar_centers = np.asarray(tar_centers, dtype=np.float32)
    idx = np.asarray(src_indices).astype(np.int64)

    tris = src_vertices[idx]                      # [N, 3, 3]
    a, b, c = tris[:, 0, :], tris[:, 1, :], tris[:, 2, :]
    normals = 0.5 * np.cross(a - b, c - b).astype(np.float32)   # [N,3]
    centers = (tris.sum(axis=1) / 3.0).astype(np.float32)       # [N,3]

    sfL, sfR = _feats13(centers)
    tfL, tfR = _feats13(tar_centers)
    snT = normals.T.astype(np.float64)
    tnT = tar_normals.T.astype(np.float64)

    featL = {"ss": sfL, "tt": tfL, "st": tfL}   # stationary (j) side
    featR = {"ss": sfR, "tt": tfR, "st": sfR}   # moving (i) side
    nrmP = {"ss": normals, "tt": tar_normals, "st": tar_normals}
    fnT = {"ss": snT, "tt": tnT, "st": snT}     # finalize (i) side

    plans = _plan2()
    in_maps = []
    fin = []   # per core: list of (visit, slot, w, fn [3,512] f64)
    for core in range(NCORES):
        visits = plans[core]
        wfeat = np.zeros((NFEAT, VISITS_PER_CORE, 4, 128), np.float32)
        rhsf = np.zeros((NFEAT, VISITS_PER_CORE, 4, 512), np.float32)
        wnrm = np.zeros((128, VISITS_PER_CORE, 12), np.float32)
        fv = []
        for vi, (m, blocks, chunks, ws) in enumerate(visits):
            for tj, blk in enumerate(blocks):
                wfeat[:, vi, tj, :] = featL[m][:, BLOCK * blk : BLOCK * (blk + 1)]
                wnrm[:, vi, 3 * tj : 3 * tj + 3] = (
                    nrmP[m][BLOCK * blk : BLOCK * (blk + 1), :])
            for cj, (ch, w) in enumerate(zip(chunks, ws)):
                rhsf[:, vi, cj, :] = featR[m][:, CHUNK * ch : CHUNK * (ch + 1)]
                fv.append((vi, cj, w,
                           fnT[m][:, CHUNK * ch : CHUNK * (ch + 1)]))
        in_maps.append({
            "wfeat": wfeat.astype(ml_dtypes.bfloat16),
            "rhsf": rhsf.astype(ml_dtypes.bfloat16),
            "wnrm": wnrm.astype(ml_dtypes.bfloat16),
        })
        fin.append(fv)

    nc = _build_nc()
    results = run_bass_kernel_spmd(nc, in_maps, list(range(NCORES))).results

    e = 0.0
    for core in range(NCORES):
        sout = np.asarray(results[core]["sout"], dtype=np.float64)
        for vi, cj, w, fn in fin[core]:
            e += w * float((sout[vi, 32 * cj : 32 * cj + 3] * fn).sum())
    return np.float32(e)
